# revision 2
# baseline (speedup 1.0000x reference)
"""AttentionHead kernel for 8x TRN2 NeuronCores (Bass/Tile).

Reference semantics (faithful quirk: attention mixes HEADS at each position):
  q = x@Wq.T+bq ; k,v likewise ; reshape [B,S,H,Dk]
  scores[b,s,h,t] = sum_d q[b,s,h,d]*k[b,s,t,d] / sqrt(D)
  attn = softmax_t(scores) ; out[b,s,h,:] = sum_t attn*v[b,s,t,:]
  final = out@Wo.T + bo

Sharding: data-parallel over the 16384 tokens (2048/core). Weights replicated.
Host prep: bf16 casts, x transposed to [D, T], W's pre-transposed to [in, out]
and packed into one wall, Wv/bv column-permuted so V comes out d-major
([s, (d,t)]) making the AV reduction innermost on DVE.

Toolchain constraint honored throughout: each instruction may carry at most
ONE semaphore wait (plus its own update), so DMAs are merged/batched so no
DMA or copy ever needs two distinct waits.
"""

import numpy as np
import ml_dtypes

import concourse.bass as bass
import concourse.mybir as mybir
from concourse import bacc
from concourse.tile import TileContext
from concourse.bass_utils import run_bass_kernel_spmd
from concourse.masks import make_identity

BF16 = ml_dtypes.bfloat16

B, S, D = 4, 4096, 1024
H, DK = 16, 64
NCORES = 8
T = B * S                 # 16384 tokens
TPC = T // NCORES         # 2048 per core
PT = 128                  # tokens per tile (partition dim)
NT = TPC // PT            # 16 tiles per core
LG = 1                    # tiles per x-load group
SG = 2                    # tiles per store group

_CACHE = {}


def _build_nc():
    nc = bacc.Bacc()
    dt = mybir.dt

    xT = nc.declare_dram_parameter("xT", [D, TPC], dt.bfloat16, isOutput=False)
    wall = nc.declare_dram_parameter("wall", [D, 4 * D], dt.bfloat16, isOutput=False)
    bias4 = nc.declare_dram_parameter("bias4", [1, 4 * D], dt.bfloat16, isOutput=False)
    out = nc.declare_dram_parameter("out", [TPC, D], dt.float32, isOutput=True)

    with TileContext(nc) as tc:
        with (
            tc.tile_pool(name="wpool", bufs=1) as wpool,
            tc.tile_pool(name="xpool", bufs=4) as xpool,
            tc.tile_pool(name="qkv", bufs=2) as qkvpool,
            tc.tile_pool(name="mid", bufs=2) as midpool,
            tc.tile_pool(name="small", bufs=3) as smallpool,
            tc.tile_pool(name="big", bufs=2) as bigpool,
            tc.tile_pool(name="ppsum", bufs=6, space="PSUM") as ppsum,
            tc.tile_pool(name="tpsum", bufs=2, space="PSUM") as tpsum,
        ):
            # ---- one-time loads (2 HWDGE DMAs total) ----
            w_sb = wpool.tile([PT, 4, 8, D], dt.bfloat16)
            wall4 = wall.rearrange("(c p) (m o) -> p m c o", p=PT, o=D)
            for m in range(4):
                nc.sync.dma_start(
                    out=w_sb[:, m : m + 1, :, :], in_=wall4[:, m : m + 1, :, :]
                )
            bias_sb = wpool.tile([1, 4 * D], dt.bfloat16)
            nc.sync.dma_start(out=bias_sb, in_=bias4[:, :])
            ones_sb = wpool.tile([1, PT], dt.bfloat16)
            nc.vector.memset(ones_sb, 1.0)
            ident = wpool.tile([PT, PT], dt.bfloat16)
            make_identity(nc, ident)
            zbias = wpool.tile([PT, 1], dt.float32)
            nc.vector.memset(zbias, 0.0)

            inv_sqrt_d = 1.0 / np.sqrt(np.float32(D))

            xt2 = None
            for grp in range(NT // SG):
                fout = bigpool.tile([PT, SG, D], dt.float32, tag="fout")
                # absorbs the WAR-vs-store wait into DVE's clock so the psum
                # copies below need only their PE wait
                nc.vector.memset(fout[:, 0, 0:1], 0.0)
                for sub in range(SG):
                    it = grp * SG + sub
                    tok = it * PT
                    if it % LG == 0:
                        xt2 = xpool.tile([PT, 8, LG * PT], dt.bfloat16, tag="xt")
                        nc.sync.dma_start(
                            out=xt2,
                            in_=xT[:, tok : tok + LG * PT].rearrange(
                                "(c p) s -> p c s", p=PT
                            ),
                        )
                    xoff = (it % LG) * PT
                    xt = xt2[:, :, xoff : xoff + PT]

                    # ---- projections q,k,v ----
                    qkv_sb = []
                    for m in range(3):
                        dst = qkvpool.tile([PT, D], dt.bfloat16, tag=f"qkv{m}")
                        for half in range(2):
                            off = half * 512
                            ps = ppsum.tile([PT, 512], dt.float32, tag="ppsum")
                            nc.tensor.matmul(
                                ps,
                                ones_sb,
                                bias_sb[:, m * D + off : m * D + off + 512],
                                start=True,
                                stop=False,
                            )
                            for c in range(8):
                                nc.tensor.matmul(
                                    ps,
                                    xt[:, c, :],
                                    w_sb[:, m, c, off : off + 512],
                                    start=False,
                                    stop=(c == 7),
                                )
                            nc.scalar.activation(dst[:, off : off + 512], ps, func=mybir.ActivationFunctionType.Copy)
                        qkv_sb.append(dst)
                    q_sb, k_sb, v_sb = qkv_sb
                    q3 = q_sb.rearrange("p (h d) -> p h d", h=H)
                    k3 = k_sb.rearrange("p (t d) -> p t d", t=H)
                    v3 = v_sb.rearrange("p (d t) -> p d t", d=DK)  # d-major

                    # ---- scores[s,h,t] = sum_d q[s,h,d]*k[s,t,d] ----
                    # processed in head PAIRS: one 4D mul covers both heads
                    scores = smallpool.tile([PT, H, H], dt.float32, tag="scores")
                    s3 = scores  # [128, 16, 16]
                    for hp in range(H // 4):
                        qb = bass.AP(
                            tensor=q_sb.tensor,
                            offset=q_sb.offset + 4 * hp * DK,
                            ap=[q_sb.ap[0], [DK, 4], [0, H], [1, DK]],
                        )  # [128, 4(h), 16(bcast t), 64]
                        kb = bass.AP(
                            tensor=k_sb.tensor,
                            offset=k_sb.offset,
                            ap=[k_sb.ap[0], [0, 4], [DK, H], [1, DK]],
                        )  # [128, 4(bcast h), 16(t), 64]
                        prod = midpool.tile([PT, 4, H, DK], dt.bfloat16, tag="prod")
                        nc.vector.tensor_mul(prod, qb, kb)
                        t32 = midpool.tile([PT, 4, H, 32], dt.bfloat16, tag="t32")
                        nc.vector.tensor_add(
                            t32, prod[:, :, :, 0:32], prod[:, :, :, 32:64]
                        )
                        t16 = midpool.tile([PT, 4, H, 16], dt.bfloat16, tag="t16")
                        nc.vector.tensor_add(
                            t16, t32[:, :, :, 0:16], t32[:, :, :, 16:32]
                        )
                        t8s = midpool.tile([PT, 4, H, 8], dt.bfloat16, tag="t8s")
                        nc.vector.tensor_add(
                            t8s, t16[:, :, :, 0:8], t16[:, :, :, 8:16]
                        )
                        nc.vector.tensor_reduce(
                            s3[:, 4 * hp : 4 * hp + 4, :], t8s,
                            axis=mybir.AxisListType.X, op=mybir.AluOpType.add,
                        )

                    # ---- softmax over t (scale by 1/sqrt(D) inside exp) ----
                    probs = smallpool.tile([PT, H, H], dt.bfloat16, tag="probs")
                    nc.scalar.activation(
                        probs, scores, func=mybir.ActivationFunctionType.Exp,
                        bias=zbias[:, 0:1], scale=float(inv_sqrt_d),
                    )
                    denom = smallpool.tile([PT, H], dt.float32, tag="denom")
                    nc.vector.tensor_reduce(
                        denom, probs, axis=mybir.AxisListType.X,
                        op=mybir.AluOpType.add,
                    )
                    rden = smallpool.tile([PT, H], dt.float32, tag="rden")
                    nc.vector.reciprocal(rden, denom)
                    rden_bf = smallpool.tile([PT, H], dt.bfloat16, tag="rdenbf")
                    nc.vector.tensor_copy(rden_bf, rden)
                    probs_n = smallpool.tile([PT, H, H], dt.bfloat16, tag="probsn")
                    rb = bass.AP(
                        tensor=rden_bf.tensor,
                        offset=rden_bf.offset,
                        ap=[rden_bf.ap[0], rden_bf.ap[-1], [0, H]],
                    )  # [128,16,16(bcast t)]
                    nc.vector.tensor_mul(probs_n, probs, rb)

                    # ---- out2[s,h,d] = sum_t probs_n[s,h,t] * v[s,t,d] ----
                    out2 = bigpool.tile([PT, D], dt.float32, tag="out2")
                    o23 = out2.rearrange("p (h d) -> p h d", h=H)
                    for hp in range(H // 4):
                        pb = bass.AP(
                            tensor=probs_n.tensor,
                            offset=probs_n.offset + 4 * hp * H,
                            ap=[probs_n.ap[0], [H, 4], [0, DK], [1, H]],
                        )  # [128, 4(h), 64(bcast d), 16]
                        vb = bass.AP(
                            tensor=v_sb.tensor,
                            offset=v_sb.offset,
                            ap=[v_sb.ap[0], [0, 4], [H, DK], [1, H]],
                        )  # [128, 4(bcast h), 64(d), 16]
                        prodv = midpool.tile([PT, 4, DK, H], dt.bfloat16, tag="prodv")
                        nc.vector.tensor_mul(prodv, pb, vb)
                        t8 = midpool.tile([PT, 4, DK, 8], dt.bfloat16, tag="t8")
                        nc.vector.tensor_add(
                            t8, prodv[:, :, :, 0:8], prodv[:, :, :, 8:16]
                        )
                        t4 = midpool.tile([PT, 4, DK, 4], dt.bfloat16, tag="t4")
                        nc.vector.tensor_add(
                            t4, t8[:, :, :, 0:4], t8[:, :, :, 4:8]
                        )
                        t2 = midpool.tile([PT, 4, DK, 2], dt.bfloat16, tag="t2")
                        nc.vector.tensor_add(
                            t2, t4[:, :, :, 0:2], t4[:, :, :, 2:4]
                        )
                        nc.vector.tensor_reduce(
                            o23[:, 4 * hp : 4 * hp + 4, :], t2,
                            axis=mybir.AxisListType.X, op=mybir.AluOpType.add,
                        )

                    # ---- transpose attn-out, final projection ----
                    ao_bf = bigpool.tile([PT, D], dt.bfloat16, tag="aobf")
                    nc.scalar.activation(ao_bf, out2, func=mybir.ActivationFunctionType.Copy)
                    aoT = bigpool.tile([PT, 8, PT], dt.bfloat16, tag="aoT")
                    for g in range(2):
                        tp = tpsum.tile([PT, 512], dt.bfloat16, tag="tpsum")
                        for j in range(4):
                            c = g * 4 + j
                            nc.tensor.transpose(
                                tp[:, j * PT : (j + 1) * PT],
                                ao_bf[:, c * PT : (c + 1) * PT],
                                ident,
                            )
                        nc.scalar.activation(
                            aoT[:, g * 4 : (g + 1) * 4, :].rearrange(
                                "p c s -> p (c s)"
                            ),
                            tp, func=mybir.ActivationFunctionType.Copy,
                        )
                    for half in range(2):
                        off = half * 512
                        ps = ppsum.tile([PT, 512], dt.float32, tag="ppsum")
                        nc.tensor.matmul(
                            ps,
                            ones_sb,
                            bias_sb[:, 3 * D + off : 3 * D + off + 512],
                            start=True,
                            stop=False,
                        )
                        for c in range(8):
                            nc.tensor.matmul(
                                ps,
                                aoT[:, c, :],
                                w_sb[:, 3, c, off : off + 512],
                                start=False,
                                stop=(c == 7),
                            )
                        nc.scalar.activation(fout[:, sub, off : off + 512], ps, func=mybir.ActivationFunctionType.Copy)
                # one batched store per group (4 total -> no HWDGE lane reuse)
                gtok = grp * SG * PT
                nc.sync.dma_start(
                    out=out[gtok : gtok + SG * PT, :].rearrange(
                        "(j p) o -> p j o", p=PT
                    ),
                    in_=fout,
                )

    nc.compile()
    return nc


def _host_prep(x, Wq, bq, Wk, bk, Wv, bv, Wo, bo):
    xt = np.ascontiguousarray(x.reshape(T, D).T).astype(BF16)  # [D, T]
    perm = (np.arange(D).reshape(H, DK).T).reshape(-1)  # perm[d*16+t] = t*64+d
    wall = np.concatenate(
        [Wq.T, Wk.T, Wv.T[:, perm], Wo.T], axis=1
    ).astype(BF16)  # [D, 4D]
    bias4 = np.concatenate([bq, bk, bv[perm], bo]).astype(BF16)[None, :]
    return xt, np.ascontiguousarray(wall), bias4


def kernel(x, Wq, bq, Wk, bk, Wv, bv, Wo, bo, _trace=False):
    x = np.asarray(x, dtype=np.float32)
    arrs = [np.asarray(a, dtype=np.float32) for a in (Wq, bq, Wk, bk, Wv, bv, Wo, bo)]
    xt, wall, bias4 = _host_prep(x, *arrs)

    if "nc" not in _CACHE:
        _CACHE["nc"] = _build_nc()
    nc = _CACHE["nc"]

    in_maps = []
    for c in range(NCORES):
        in_maps.append(
            {
                "wall": wall,
                "bias4": bias4,
                "xT": np.ascontiguousarray(xt[:, c * TPC : (c + 1) * TPC]),
            }
        )

    _CACHE["in_maps"] = in_maps
    res = run_bass_kernel_spmd(nc, in_maps, core_ids=list(range(NCORES)), trace=_trace)
    _CACHE["last_result"] = res
    out = np.concatenate([res.results[c]["out"] for c in range(NCORES)], axis=0)
    return out.reshape(B, S, D)

